# revision 1
# baseline (speedup 1.0000x reference)
"""Trainium2 Bass kernel for the CustomRNN problem.

Reference computation (per time step t over T=1024):
    h = tanh(h @ W2.T + x_t[:, None] @ W1.T + b_h)      # h: [B, H]
    y_t = h @ W3.T                                       # [B, O]

Strategy (data-parallel over batch, 8 cores x 16 rows each):
  * The recurrence runs in TRANSPOSED form on-chip: state is g = h^T with
    layout [H, B_loc] (H=512 -> 4 partition tiles of [128, 16]).  Each step,
    for each output chunk m:
        z^T[m]  = sum_k W2T_tile[k, m].T @ g[k]       (PE, accumulate in PSUM)
        z^T[m] += [W1[m] | b_h[m]].T @ [x_t; 1]       (PE, K=2 matmul -> fuses
                                                       the input and bias terms
                                                       into the same PSUM group)
        g'[m]   = tanh(z^T[m])                        (ACT, PSUM -> SBUF)
    The output layout of the matmul ([H_out, B]) is exactly the input layout
    needed by the next step, so no per-step transpose is required, and the
    only cross-engine hop on the serial critical path is the single ACT.
  * g'[m] is written directly into a double-buffered history buffer hist[m];
    every S=32 steps the y output for the window is computed as
        y^T = sum_k W3T[k].T @ hist[k][:, window]     ([O, S*B_loc] per window)
    and DMA'd out.  Host code undoes the transpose at the end.
"""

import sys

for _p in ("/opt/trn_rl_repo",):
    if _p not in sys.path:
        sys.path.insert(0, _p)

import numpy as np

import concourse.bacc as bacc
import concourse.bass as bass
import concourse.mybir as mybir
import concourse.tile as tile
from concourse.bass_utils import run_bass_kernel_spmd

# Problem constants (hardcoded per contract).
B, T, H, O = 128, 1024, 512, 10
NCORES = 8
BLOC = B // NCORES        # 16 batch rows per core
P = 128                   # partition dim
KC = H // P               # 4 chunks of the hidden dim
S = 32                    # y-window length (N = S*BLOC = 512 moving cols)

F32 = mybir.dt.float32


def build_nc(t_steps: int = T, mm_dt=mybir.dt.float16, reps: int = 1):
    """Build the single-core Bass program (same program runs SPMD on 8 cores).

    reps > 1 repeats the whole recurrence (identical I/O, multiplied
    compute) — used by the benchmark harness to measure device execution
    time differentially, cancelling RPC/transfer overhead.
    """
    assert t_steps % S == 0

    nc = bacc.Bacc("TRN2", target_bir_lowering=False)
    # x_aug[0, t*16+b] = x[b, t], x_aug[1, :] = 1.0
    xa_d = nc.dram_tensor("x_aug", [2, t_steps * BLOC], mm_dt, kind="ExternalInput")
    w2t_d = nc.dram_tensor("w2t", [H, H], mm_dt, kind="ExternalInput")
    # waug[0, :] = W1, waug[1, :] = b_h  (both in H-major order)
    wa_d = nc.dram_tensor("waug", [2, H], mm_dt, kind="ExternalInput")
    w3t_d = nc.dram_tensor("w3t", [H, O], mm_dt, kind="ExternalInput")
    yt_d = nc.dram_tensor("yT", [O, t_steps * BLOC], F32, kind="ExternalOutput")

    def slot_col(t):
        # double-buffered hist slot for step t (parity of its window)
        return (((t // S) % 2) * S + (t % S)) * BLOC

    with tile.TileContext(nc) as tc:
        with (
            tc.tile_pool(name="const", bufs=1) as const,
            tc.tile_pool(name="zpsum", bufs=6, space="PSUM") as zpool,
            tc.tile_pool(name="ypsum", bufs=2, space="PSUM") as ypool,
            tc.tile_pool(name="ysb", bufs=2) as yspool,
        ):
            # --- persistent SBUF tensors -------------------------------
            # W2^T tiles, k-major: chunk (k, m) at columns (k*KC + m)*P
            w2sb = const.tile([P, KC * KC * P], mm_dt, tag="w2sb")
            for k in range(KC):
                nc.sync.dma_start(
                    w2sb[:, k * KC * P : (k + 1) * KC * P],
                    w2t_d[k * P : (k + 1) * P, :],
                )
            w3sb = const.tile([P, KC * O], mm_dt, tag="w3sb")
            for k in range(KC):
                nc.sync.dma_start(
                    w3sb[:, k * O : (k + 1) * O], w3t_d[k * P : (k + 1) * P, :]
                )
            wasb = const.tile([2, H], mm_dt, tag="wasb")
            nc.sync.dma_start(wasb[:], wa_d[:])
            xasb = const.tile([2, t_steps * BLOC], mm_dt, tag="xasb")
            nc.sync.dma_start(xasb[:], xa_d[:])

            hist = [
                const.tile([P, 2 * S * BLOC], mm_dt, tag=f"hist{k}", name=f"hist{k}")
                for k in range(KC)
            ]
            zeros = const.tile([P, BLOC], mm_dt, tag="zeros")
            nc.vector.memset(zeros[:], 0.0)

            # --- the recurrence ---------------------------------------
            for _rep in range(reps):
              for t in range(t_steps):
                sc = slot_col(t)
                xa = xasb[:, t * BLOC : (t + 1) * BLOC]
                for m in range(KC):
                    zp = zpool.tile([P, BLOC], F32, tag="zp")
                    # input + bias term first: z = [W1|bh].T @ [x_t; 1]
                    nc.tensor.matmul(
                        zp[:],
                        wasb[:, m * P : (m + 1) * P],
                        xa,
                        start=True,
                        stop=False,
                    )
                    for k in range(KC):
                        if t == 0:
                            rhs = zeros[:]
                        else:
                            pc = slot_col(t - 1)
                            rhs = hist[k][:, pc : pc + BLOC]
                        nc.tensor.matmul(
                            zp[:],
                            w2sb[:, (k * KC + m) * P : (k * KC + m + 1) * P],
                            rhs,
                            start=False,
                            stop=(k == KC - 1),
                        )
                    # g'[m] = tanh(z) -> hist slot (casts to mm_dt)
                    nc.scalar.activation(
                        hist[m][:, sc : sc + BLOC],
                        zp[:],
                        mybir.ActivationFunctionType.Tanh,
                    )

                # --- y output for a finished window -------------------
                if t % S == S - 1:
                    w = t // S
                    par = w % 2
                    yp = ypool.tile([O, S * BLOC], F32, tag="yp")
                    for k in range(KC):
                        nc.tensor.matmul(
                            yp[:],
                            w3sb[:, k * O : (k + 1) * O],
                            hist[k][:, par * S * BLOC : (par + 1) * S * BLOC],
                            start=(k == 0),
                            stop=(k == KC - 1),
                        )
                    ys = yspool.tile([O, S * BLOC], F32, tag="ys")
                    nc.vector.tensor_copy(ys[:], yp[:])
                    nc.sync.dma_start(
                        yt_d[:, w * S * BLOC : (w + 1) * S * BLOC], ys[:]
                    )
    nc.compile()
    return nc


def _np_mm_dtype(mm_dt):
    return {F32: np.float32, mybir.dt.float16: np.float16}[mm_dt]


def make_in_maps(x, W1, W2, W3, b_h, t_steps: int = T, mm_dt=mybir.dt.float16):
    x = np.asarray(x, np.float32)[:, :t_steps]
    W1 = np.asarray(W1, np.float32)
    W2 = np.asarray(W2, np.float32)
    W3 = np.asarray(W3, np.float32)
    b_h = np.asarray(b_h, np.float32)
    mdt = _np_mm_dtype(mm_dt)

    w2t = np.ascontiguousarray(W2.T).astype(mdt)          # [H, H]
    w3t = np.ascontiguousarray(W3.T).astype(mdt)          # [H, O]
    waug = np.stack([W1.reshape(-1), b_h]).astype(mdt)    # [2, H]

    in_maps = []
    for c in range(NCORES):
        xs = x[c * BLOC : (c + 1) * BLOC]                  # [16, t]
        xaug = np.empty((2, t_steps * BLOC), mdt)
        xaug[0] = np.ascontiguousarray(xs.T).reshape(-1)   # (t, b) order
        xaug[1] = 1.0
        in_maps.append({"x_aug": xaug, "w2t": w2t, "w3t": w3t, "waug": waug})
    return in_maps


def gather_output(results, t_steps: int = T):
    out = np.empty((B, t_steps, O), np.float32)
    for c in range(NCORES):
        yt = results[c]["yT"]  # [O, t*16] in (o, t, b) order
        out[c * BLOC : (c + 1) * BLOC] = (
            yt.reshape(O, t_steps, BLOC).transpose(2, 1, 0)
        )
    return out


_NC_CACHE = {}

MM_DT = mybir.dt.float16  # matmul dtype for W2/W3/state (accumulation is f32)


def kernel(x, W1, W2, W3, b_h):
    key = (T, MM_DT)
    if key not in _NC_CACHE:
        _NC_CACHE[key] = build_nc(T, MM_DT)
    nc = _NC_CACHE[key]
    in_maps = make_in_maps(x, W1, W2, W3, b_h, T, MM_DT)
    res = run_bass_kernel_spmd(nc, in_maps, core_ids=list(range(NCORES))).results
    return gather_output(res, T)

